# revision 25
# baseline (speedup 1.0000x reference)
"""Trainium2 Bass kernel for nn_Attention (dense transformer block:
qkv proj + RoPE + causal attention + out proj), tensor-parallel over
8 NeuronCores: core c handles batch b=c//2, head-group g=c%2 (8 heads).

Self-contained: hardcodes all shapes; host preps transposed/permuted
shards, device computes partial y per core, host sums head-group pairs
and adds the output bias.

All-bf16 datapath (fp8 was tried and rejected: e4m3 quantization is
~3.6% relative per element and random-sign dot products don't average
it away, blowing the 2e-2 budget). RoPE runs on DVE in 2x bf16 mode
with the even/odd swap done by stream_shuffle (16-granularity pair
layout prepared host-side). Softmax exp on the Activation engine is
the attention-phase bound; denominators come free via a ones-row
appended to V.
"""

from contextlib import ExitStack

import numpy as np

import concourse.bass as bass
import concourse.tile as tile
from concourse import bacc, mybir
from concourse.bass import ds, ts
from concourse.bass_utils import run_bass_kernel_spmd

B, S, D, H, DH = 4, 2048, 1024, 16, 64
HL = 8          # heads per core
INNER = H * DH  # 1024
KC = D // 128   # 8 contraction chunks
NT = S // 128   # 16 token tiles
F32 = mybir.dt.float32
F32R = mybir.dt.float32r
BF16 = mybir.dt.bfloat16
W_SCALE = 1.0  # no pre-scaling needed at bf16


def _pieces(cw):
    """split a psum-tile column span into single-bank matmul pieces"""
    out = [(i * 512, 512) for i in range(cw // 512)]
    if cw % 512:
        out.append((cw - cw % 512, cw % 512))
    return out


def build_kernel(nc, phases=3, repeats=1):
    xT = nc.dram_tensor("xT", [D, S], BF16, kind="ExternalInput").ap()
    wq = nc.dram_tensor("wq", [D, HL * DH], BF16, kind="ExternalInput").ap()
    wk = nc.dram_tensor("wk", [D, HL * DH], BF16, kind="ExternalInput").ap()
    wv = nc.dram_tensor("wv", [D, HL * DH], BF16, kind="ExternalInput").ap()
    wo = nc.dram_tensor("wo", [HL * DH, D], BF16, kind="ExternalInput").ap()
    cc = nc.dram_tensor("cc", [128, S], BF16, kind="ExternalInput").ap()
    ssw = nc.dram_tensor("ssw", [128, S], BF16, kind="ExternalInput").ap()
    tri = nc.dram_tensor("tri", [128, 128], BF16, kind="ExternalInput").ap()
    y = nc.dram_tensor("y", [S, D], BF16, kind="ExternalOutput").ap()

    with tile.TileContext(nc) as tc:
        for _ in range(repeats):
            _kernel_body(nc, tc, xT, wq, wk, wv, wo, cc, ssw, tri, y,
                         phases)
    return nc


def _kernel_body(nc, tc, xT, wq, wk, wv, wo, cc, ssw, tri, y, phases=3):
    EXP = mybir.ActivationFunctionType.Exp
    SCALE = 1.0 / np.sqrt(DH)

    with ExitStack() as top:
        opool = top.enter_context(tc.tile_pool(name="opool", bufs=1))
        ot = None  # [128, 4, S] bf16: partition=head-pair dims, dim1=pair idx
        wo_sb = [opool.tile([128, D], BF16, tag=f"wo{k}", name=f"wo{k}")
                 for k in range(4)]

        with ExitStack() as mid:
            qkp = mid.enter_context(tc.tile_pool(name="qkt", bufs=1))
            vpool = mid.enter_context(tc.tile_pool(name="vpool", bufs=1))
            qkt = [qkp.tile([128, S], BF16, tag=f"qkt{t}", name=f"qkt{t}")
                   for t in range(8)]
            vsb = vpool.tile([128, NT, HL, DH + 1], BF16, tag="vsb",
                             name="vsb")

            # ---------------- phase B: projections + rope -----------------
            with ExitStack() as ph:
                consts = ph.enter_context(tc.tile_pool(name="consts", bufs=1))
                xtp = ph.enter_context(tc.tile_pool(name="xtp", bufs=2))
                rtmp = ph.enter_context(tc.tile_pool(name="rtmp", bufs=6))
                psqk = ph.enter_context(
                    tc.tile_pool(name="psqk", bufs=3, space="PSUM"))
                psv = ph.enter_context(
                    tc.tile_pool(name="psv", bufs=2, space="PSUM"))

                # constants + first x half, in consumption order: the sync
                # queue drains in issue order, so put the first-needed
                # tensors first (wq -> xh0 -> rope tables -> wv -> wk),
                # and wo (out projection) last.
                wq_m = [consts.tile([128, KC, 128], BF16, tag=f"wq{m}",
                                    name=f"wq{m}") for m in range(4)]
                nc.sync.dma_start(
                    wq_m[0][:],
                    wq.rearrange("(k p) n -> p k n", p=128)[:, :, ts(0, 128)])
                xh0a = xtp.tile([128, KC // 2, 1024], BF16, tag="xha",
                                name="xha")
                nc.sync.dma_start(
                    xh0a[:],
                    xT.rearrange("(k p) s -> p k s", p=128)[:, ds(0, 4),
                                                            ds(0, 1024)])
                xh0b = xtp.tile([128, KC // 2, 1024], BF16, tag="xhb",
                                name="xhb")
                nc.sync.dma_start(
                    xh0b[:],
                    xT.rearrange("(k p) s -> p k s", p=128)[:, ds(4, 4),
                                                            ds(0, 1024)])
                for m in range(1, 4):
                    nc.sync.dma_start(
                        wq_m[m][:],
                        wq.rearrange("(k p) n -> p k n", p=128)[:, :,
                                                                ts(m, 128)])
                cc_sb = consts.tile([128, S], BF16, tag="cc", name="cc")
                nc.sync.dma_start(cc_sb[:], cc[:, :])
                ssw_sb = consts.tile([128, S], BF16, tag="ssw", name="ssw")
                nc.sync.dma_start(ssw_sb[:], ssw[:, :])
                wv_sb = consts.tile([128, KC, 512], BF16, tag="wv", name="wv")
                nc.sync.dma_start(
                    wv_sb[:], wv.rearrange("(k p) n -> p k n", p=128))
                wk_sb = consts.tile([128, KC, 512], BF16, tag="wk", name="wk")
                nc.sync.dma_start(
                    wk_sb[:], wk.rearrange("(k p) n -> p k n", p=128))
                tri_sb = consts.tile([128, 128], BF16, tag="tri", name="tri")
                nc.sync.dma_start(tri_sb[:], tri[:, :])
                for k in range(4):
                    nc.sync.dma_start(wo_sb[k][:], wo[ts(k, 128), :])

                nc.gpsimd.memset(vsb[:, :, :, DH], 1.0)

                for half in range(2):
                    hs = ds(half * 1024, 1024)
                    if half == 0:
                        def xck(k):
                            return (xh0a, k) if k < 4 else (xh0b, k - 4)
                    else:
                        xh = xtp.tile([128, KC, 1024], BF16, tag="xh", name="xh")
                        nc.sync.dma_start(
                            xh[:],
                            xT.rearrange("(k p) s -> p k s", p=128)[:, :, hs])

                        def xck(k, _xh=xh):
                            return (_xh, k)
                    # q/k projections + rope, interleaved with the v
                    # projection
                    for t in range(8):
                        m = t % 4
                        ps = psqk.tile([128, 1024], F32, tag="psqk")
                        for k in range(KC):
                            xt_k, lk = xck(k)
                            w_ap = (wq_m[m][:, k, :] if t < 4
                                    else wk_sb[:, k, ts(m, 128)])
                            for p2 in range(2):
                                nc.tensor.matmul(
                                    ps[:, ts(p2, 512)],
                                    w_ap,
                                    xt_k[:, lk, ts(p2, 512)],
                                    start=(k == 0), stop=(k == KC - 1))
                        # rope on DVE in bf16: out = t*CC + swap32(t*SSsw)
                        qkb = rtmp.tile([128, 1024], BF16, tag="qkb")
                        nc.scalar.copy(qkb[:], ps[:])
                        nc.vector.tensor_mul(qkt[t][:, hs], qkb[:], cc_sb[:, hs])
                        v2 = rtmp.tile([128, 1024], BF16, tag="v2")
                        nc.vector.tensor_mul(v2[:], qkb[:], ssw_sb[:, hs])
                        v2s = rtmp.tile([128, 1024], BF16, tag="v2s", name="v2s")
                        nc.vector.stream_shuffle(
                            v2s[:], v2[:], mask=[i ^ 16 for i in range(32)])
                        nc.gpsimd.tensor_tensor(
                            qkt[t][:, hs], qkt[t][:, hs], v2s[:],
                            op=mybir.AluOpType.add)
                        # v projection tile for this slot
                        tt = half * 8 + t
                        psV = psv.tile([128, 512], F32, tag="psv")
                        for k in range(KC):
                            xt_k, lk = xck(k)
                            nc.tensor.matmul(
                                psV[:],
                                xt_k[:, lk, ds(t * 128, 128)],
                                wv_sb[:, k, :],
                                start=(k == 0), stop=(k == KC - 1))
                        nc.scalar.copy(
                            vsb[:, tt, :, 0:DH],
                            psV[:].rearrange("p (h d) -> p h d", h=HL))

            # ---------------- attention ----------------------------------
            if phases < 2:
                return
            with ExitStack() as ph:
                ppool = ph.enter_context(tc.tile_pool(name="ppool", bufs=8))
                lpool = ph.enter_context(tc.tile_pool(name="lpool", bufs=4))
                pssc = ph.enter_context(
                    tc.tile_pool(name="pssc", bufs=2, space="PSUM"))
                psav = ph.enter_context(
                    tc.tile_pool(name="psav", bufs=2, space="PSUM"))

                ot = opool.tile([128, 4, S], BF16, tag="ot", name="ot")
                for h in range(HL):
                    ht, hb = h // 2, 64 * (h % 2)
                    q_ap = qkt[ht][ds(hb, 64), :]
                    k_ap = qkt[4 + ht][ds(hb, 64), :]
                    for qh in range(2):
                        q0, q1 = 1024 * qh, 1024 * (qh + 1)
                        pav = psav.tile([DH + 1, 1024], F32, tag="pav")
                        for j in range(8 * (qh + 1)):
                            gs = max(q0, 128 * j)     # first valid q col
                            cw = q1 - gs
                            ps = pssc.tile([128, cw], F32, tag="sc")
                            for (po, pw) in _pieces(cw):
                                nc.tensor.matmul(
                                    ps[:, ds(po, pw)],
                                    (k_ap[:, ds(128 * j, 128)]),
                                    (q_ap[:, ds(gs + po, pw)]),
                                    start=True, stop=True)
                            pj = ppool.tile([128, cw], BF16, tag="P")
                            nc.scalar.activation(pj[:], ps[:], EXP, scale=SCALE)
                            if gs == 128 * j:
                                # diagonal block: causal-mask first 128 cols
                                nc.gpsimd.affine_select(
                                    out=pj[:, 0:128], in_=pj[:, 0:128],
                                    compare_op=mybir.AluOpType.is_ge, fill=0.0,
                                    base=0, pattern=[[1, 128]],
                                    channel_multiplier=-1)
                            for c in range(max(2 * qh, j // 4), 2 * qh + 2):
                                cs = max(512 * c, 128 * j)
                                w = 512 * (c + 1) - cs
                                nc.tensor.matmul(
                                    pav[:, ds(cs - q0, w)],
                                    (vsb[:, j, h, :]),
                                    (pj[:, ds(cs - gs, w)]),
                                    start=(j == 0),
                                    stop=(j == min(8 * (qh + 1) - 1, 4 * c + 3)))
                        # normalize: ot rows = pav[:64] / l, l = pav[64].
                        # reciprocal is lane-local (psum lane 64 -> sbuf
                        # lane 64); the hw broadcast ucode reads partition
                        # 0, so DMA the row there first.
                        qsl = ds(q0, 1024)
                        lr = lpool.tile([128, 1024], F32, tag="lr")
                        nc.vector.reciprocal(lr[ds(64, 1), :], pav[ds(DH, 1), :])
                        nc.sync.dma_start(lr[ds(0, 1), :], lr[ds(64, 1), :])
                        rb = lpool.tile([64, 1024], F32, tag="rb")
                        nc.gpsimd.partition_broadcast(rb[:], lr[ds(0, 1), :],
                                                      channels=64)
                        if h % 2 == 0:
                            nc.vector.tensor_mul(
                                ot[ds(0, 64), ht, qsl], pav[ds(0, DH), :], rb[:])
                        else:
                            ott = lpool.tile([64, 1024], BF16, tag="ott")
                            nc.vector.tensor_mul(ott[:], pav[ds(0, DH), :], rb[:])
                            nc.sync.dma_start(ot[ds(64, 64), ht, qsl], ott[:])

        # ---------------- out projection ---------------------------------
        if phases < 3:
            return
        with ExitStack() as ph:
            ypool = ph.enter_context(tc.tile_pool(name="ypool", bufs=4))
            psy = ph.enter_context(
                tc.tile_pool(name="psy", bufs=4, space="PSUM"))
            for tt in range(NT):
                ps = psy.tile([128, D], F32, tag="psy")
                for k in range(4):
                    for half in range(2):
                        nc.tensor.matmul(
                            ps[:, ts(half, 512)],
                            (ot[:, k, ts(tt, 128)]),
                            (wo_sb[k][:, ts(half, 512)]),
                            start=(k == 0), stop=(k == 3))
                ysb = ypool.tile([128, D], BF16, tag="y")
                # alternate the psum->sbuf drain between two engines so
                # the copy never gates the matmul pipeline (gpsimd cannot
                # read PSUM on hw)
                if tt % 2 == 0:
                    nc.scalar.copy(ysb[:], ps[:])
                else:
                    nc.vector.tensor_copy(ysb[:], ps[:])
                nc.sync.dma_start(y[ts(tt, 128), :], ysb[:])


# ---------------- host side ------------------------------------------------

def _rope_tables():
    # pair layout per 64-row head block, 16-granularity so the rope
    # swap is a within-quadrant stream_shuffle with mask i^16:
    # rows [32q+0..15] = even dims of pairs 16q..16q+15,
    # rows [32q+16..31] = odd dims of the same pairs
    i = np.arange(DH // 2, dtype=np.float32)
    thetas = np.power(np.float32(10000.0), -2.0 * (i - 1.0) / DH)
    vals = thetas[:, None].astype(np.float32) * \
        np.arange(S, dtype=np.float32)[None, :]
    cos32 = np.cos(vals).astype(np.float32)   # [32 pairs, S]
    sin32 = np.sin(vals).astype(np.float32)
    cc64 = np.concatenate([cos32[0:16], cos32[0:16],
                           cos32[16:32], cos32[16:32]], axis=0)
    ss64 = np.concatenate([sin32[0:16], -sin32[0:16],
                           sin32[16:32], -sin32[16:32]], axis=0)
    CC = np.tile(cc64, (2, 1))
    SSsw = np.tile(ss64, (2, 1))
    return np.ascontiguousarray(CC), np.ascontiguousarray(SSsw)


def _qk_col_perm(g):
    cols = []
    for m in range(4):
        for hh in (2 * m, 2 * m + 1):
            hg = HL * g + hh
            for q in (0, 1):
                for eo in (0, 1):
                    cols += [hg * DH + 2 * (16 * q + i) + eo
                             for i in range(16)]
    return np.array(cols)


_CACHE = {}


def _get_module(repeats=1):
    key = f"nc{repeats}"
    if key not in _CACHE:
        nc = bacc.Bacc("TRN2", target_bir_lowering=False, debug=False,
                       num_devices=8)
        build_kernel(nc, repeats=repeats)
        nc.compile()
        _CACHE[key] = nc
    return _CACHE[key]


def make_in_maps(x, Wqkv, Wout):
    b16 = mybir.dt.np(BF16)
    x = np.ascontiguousarray(np.asarray(x, np.float32))
    Wqkv = np.asarray(Wqkv, np.float32)
    Wout = np.asarray(Wout, np.float32)
    CC, SSsw = _rope_tables()
    cc_t = np.ascontiguousarray(CC).astype(b16)
    ssw_t = np.ascontiguousarray(SSsw).astype(b16)
    # tri[k, q] = 1 where q >= k (valid causal positions in a diagonal
    # 128x128 block)
    tri_t = np.ascontiguousarray(
        np.triu(np.ones((128, 128), np.float32))).astype(b16)

    shard = {}
    for g in range(2):
        perm = _qk_col_perm(g)
        vcols = np.arange(HL * g * DH, HL * (g + 1) * DH)
        shard[g] = dict(
            wq=np.ascontiguousarray(
                Wqkv[:, 0 * INNER:1 * INNER][:, perm]).astype(b16),
            wk=np.ascontiguousarray(
                Wqkv[:, 1 * INNER:2 * INNER][:, perm]).astype(b16),
            wv=np.ascontiguousarray(
                Wqkv[:, 2 * INNER:3 * INNER][:, vcols]).astype(b16),
            wo=np.ascontiguousarray(Wout[vcols, :]).astype(b16),
        )
    in_maps = []
    for c in range(8):
        b, g = c // 2, c % 2
        in_maps.append(dict(
            xT=np.ascontiguousarray(x[b].T).astype(b16),
            cc=cc_t, ssw=ssw_t, tri=tri_t, **shard[g]))
    return in_maps


def kernel(x, Wqkv, Wout, bout):
    bout = np.asarray(bout, np.float32)
    nc = _get_module()
    in_maps = make_in_maps(x, Wqkv, Wout)
    res = run_bass_kernel_spmd(nc, in_maps, core_ids=list(range(8)))
    ys = [np.asarray(r["y"], np.float32) for r in res.results]
    out = np.stack([(ys[2 * b] + ys[2 * b + 1]) * (1.0 / W_SCALE) + bout
                    for b in range(B)])
    return out.astype(np.float32)


# revision 26
# speedup vs baseline: 1.2205x; 1.2205x over previous
"""Trainium2 Bass kernel for nn_Attention (dense transformer block:
qkv proj + RoPE + causal attention + out proj), tensor-parallel over
8 NeuronCores: core c handles batch b=c//2, head-group g=c%2 (8 heads).

Self-contained: hardcodes all shapes; host preps transposed/permuted
shards, device computes partial y per core, host sums head-group pairs
and adds the output bias.

All-bf16 datapath (fp8 was tried and rejected: e4m3 quantization is
~3.6% relative per element and random-sign dot products don't average
it away, blowing the 2e-2 budget). RoPE runs on DVE in 2x bf16 mode
with the even/odd swap done by stream_shuffle (16-granularity pair
layout prepared host-side). Softmax exp on the Activation engine is
the attention-phase bound; denominators come free via a ones-row
appended to V.
"""

from contextlib import ExitStack

import numpy as np

import concourse.bass as bass
import concourse.tile as tile
from concourse import bacc, mybir
from concourse.bass import ds, ts
from concourse.bass_utils import run_bass_kernel_spmd

B, S, D, H, DH = 4, 2048, 1024, 16, 64
HL = 8          # heads per core
INNER = H * DH  # 1024
KC = D // 128   # 8 contraction chunks
NT = S // 128   # 16 token tiles
F32 = mybir.dt.float32
F32R = mybir.dt.float32r
BF16 = mybir.dt.bfloat16
W_SCALE = 1.0  # no pre-scaling needed at bf16


def _pieces(cw):
    """split a psum-tile column span into single-bank matmul pieces"""
    out = [(i * 512, 512) for i in range(cw // 512)]
    if cw % 512:
        out.append((cw - cw % 512, cw % 512))
    return out


def build_kernel(nc, phases=3, repeats=1):
    xT = nc.dram_tensor("xT", [D, S], BF16, kind="ExternalInput").ap()
    wq = nc.dram_tensor("wq", [D, HL * DH], BF16, kind="ExternalInput").ap()
    wk = nc.dram_tensor("wk", [D, HL * DH], BF16, kind="ExternalInput").ap()
    wv = nc.dram_tensor("wv", [D, HL * DH], BF16, kind="ExternalInput").ap()
    wo = nc.dram_tensor("wo", [HL * DH, D], BF16, kind="ExternalInput").ap()
    cc = nc.dram_tensor("cc", [128, S], BF16, kind="ExternalInput").ap()
    ssw = nc.dram_tensor("ssw", [128, S], BF16, kind="ExternalInput").ap()
    tri = nc.dram_tensor("tri", [128, 128], BF16, kind="ExternalInput").ap()
    y = nc.dram_tensor("y", [S, D], BF16, kind="ExternalOutput").ap()

    with tile.TileContext(nc) as tc:
        for _ in range(repeats):
            _kernel_body(nc, tc, xT, wq, wk, wv, wo, cc, ssw, tri, y,
                         phases)
    return nc


def _kernel_body(nc, tc, xT, wq, wk, wv, wo, cc, ssw, tri, y, phases=3):
    EXP = mybir.ActivationFunctionType.Exp
    SCALE = 1.0 / np.sqrt(DH)

    with ExitStack() as top:
        opool = top.enter_context(tc.tile_pool(name="opool", bufs=1))
        ot = None  # [128, 4, S] bf16: partition=head-pair dims, dim1=pair idx
        wo_sb = [opool.tile([128, D], BF16, tag=f"wo{k}", name=f"wo{k}")
                 for k in range(4)]

        with ExitStack() as mid:
            qkp = mid.enter_context(tc.tile_pool(name="qkt", bufs=1))
            vpool = mid.enter_context(tc.tile_pool(name="vpool", bufs=1))
            qkt = [qkp.tile([128, S], BF16, tag=f"qkt{t}", name=f"qkt{t}")
                   for t in range(8)]
            vsb = vpool.tile([128, NT, HL, DH + 1], BF16, tag="vsb",
                             name="vsb")

            # ---------------- phase B: projections + rope -----------------
            with ExitStack() as ph:
                consts = ph.enter_context(tc.tile_pool(name="consts", bufs=1))
                xtp = ph.enter_context(tc.tile_pool(name="xtp", bufs=2))
                rtmp = ph.enter_context(tc.tile_pool(name="rtmp", bufs=4))
                psqk = ph.enter_context(
                    tc.tile_pool(name="psqk", bufs=3, space="PSUM"))
                psv = ph.enter_context(
                    tc.tile_pool(name="psv", bufs=2, space="PSUM"))

                # constants + first x half, in consumption order: the sync
                # queue drains in issue order, so put the first-needed
                # tensors first (wq -> xh0 -> rope tables -> wv -> wk),
                # and wo (out projection) last.
                wq_sb = consts.tile([128, KC, 512], BF16, tag="wq", name="wq")
                nc.sync.dma_start(
                    wq_sb[:], wq.rearrange("(k p) n -> p k n", p=128))
                xh0 = xtp.tile([128, KC, 1024], BF16, tag="xh", name="xh")
                nc.sync.dma_start(
                    xh0[:],
                    xT.rearrange("(k p) s -> p k s", p=128)[:, :, ds(0, 1024)])
                cc_sb = consts.tile([128, S], BF16, tag="cc", name="cc")
                nc.sync.dma_start(cc_sb[:], cc[:, :])
                ssw_sb = consts.tile([128, S], BF16, tag="ssw", name="ssw")
                nc.sync.dma_start(ssw_sb[:], ssw[:, :])
                wv_sb = consts.tile([128, KC, 512], BF16, tag="wv", name="wv")
                nc.sync.dma_start(
                    wv_sb[:], wv.rearrange("(k p) n -> p k n", p=128))
                wk_sb = consts.tile([128, KC, 512], BF16, tag="wk", name="wk")
                nc.sync.dma_start(
                    wk_sb[:], wk.rearrange("(k p) n -> p k n", p=128))
                tri_sb = consts.tile([128, 128], BF16, tag="tri", name="tri")
                nc.sync.dma_start(tri_sb[:], tri[:, :])
                for k in range(4):
                    nc.sync.dma_start(wo_sb[k][:], wo[ts(k, 128), :])

                nc.gpsimd.memset(vsb[:, :, :, DH], 1.0)

                for half in range(2):
                    hs = ds(half * 1024, 1024)
                    if half == 0:
                        xh = xh0
                    else:
                        xh = xtp.tile([128, KC, 1024], BF16, tag="xh", name="xh")
                        nc.sync.dma_start(
                            xh[:],
                            xT.rearrange("(k p) s -> p k s", p=128)[:, :, hs])
                    # q/k projections + rope, interleaved with the v
                    # projection
                    for t in range(8):
                        wsrc = wq_sb if t < 4 else wk_sb
                        m = t % 4
                        ps = psqk.tile([128, 1024], F32, tag="psqk")
                        for k in range(KC):
                            for p2 in range(2):
                                nc.tensor.matmul(
                                    ps[:, ts(p2, 512)],
                                    wsrc[:, k, ts(m, 128)],
                                    xh[:, k, ts(p2, 512)],
                                    start=(k == 0), stop=(k == KC - 1))
                        # rope on DVE in bf16: out = t*CC + swap32(t*SSsw)
                        qkb = rtmp.tile([128, 1024], BF16, tag="qkb")
                        nc.scalar.copy(qkb[:], ps[:])
                        nc.vector.tensor_mul(qkt[t][:, hs], qkb[:], cc_sb[:, hs])
                        v2 = rtmp.tile([128, 1024], BF16, tag="v2")
                        nc.vector.tensor_mul(v2[:], qkb[:], ssw_sb[:, hs])
                        v2s = rtmp.tile([128, 1024], BF16, tag="v2s", name="v2s")
                        nc.vector.stream_shuffle(
                            v2s[:], v2[:], mask=[i ^ 16 for i in range(32)])
                        nc.gpsimd.tensor_tensor(
                            qkt[t][:, hs], qkt[t][:, hs], v2s[:],
                            op=mybir.AluOpType.add)
                        # v projection tile for this slot
                        tt = half * 8 + t
                        psV = psv.tile([128, 512], F32, tag="psv")
                        for k in range(KC):
                            nc.tensor.matmul(
                                psV[:],
                                xh[:, k, ds(t * 128, 128)],
                                wv_sb[:, k, :],
                                start=(k == 0), stop=(k == KC - 1))
                        nc.scalar.copy(
                            vsb[:, tt, :, 0:DH],
                            psV[:].rearrange("p (h d) -> p h d", h=HL))

            # ---------------- attention ----------------------------------
            if phases < 2:
                return
            with ExitStack() as ph:
                ppool = ph.enter_context(tc.tile_pool(name="ppool", bufs=5))
                lpool = ph.enter_context(tc.tile_pool(name="lpool", bufs=2))
                pssc = ph.enter_context(
                    tc.tile_pool(name="pssc", bufs=2, space="PSUM"))
                psav = ph.enter_context(
                    tc.tile_pool(name="psav", bufs=2, space="PSUM"))

                ot = opool.tile([128, 4, S], BF16, tag="ot", name="ot")
                for h in range(HL):
                    ht, hb = h // 2, 64 * (h % 2)
                    q_ap = qkt[ht][ds(hb, 64), :]
                    k_ap = qkt[4 + ht][ds(hb, 64), :]
                    for qh in range(2):
                        q0, q1 = 1024 * qh, 1024 * (qh + 1)
                        pav = psav.tile([DH + 1, 1024], F32, tag="pav")
                        for j in range(8 * (qh + 1)):
                            gs = max(q0, 128 * j)     # first valid q col
                            cw = q1 - gs
                            ps = pssc.tile([128, cw], F32, tag="sc")
                            for (po, pw) in _pieces(cw):
                                nc.tensor.matmul(
                                    ps[:, ds(po, pw)],
                                    (k_ap[:, ds(128 * j, 128)]),
                                    (q_ap[:, ds(gs + po, pw)]),
                                    start=True, stop=True)
                            pj = ppool.tile([128, cw], BF16, tag="P")
                            nc.scalar.activation(pj[:], ps[:], EXP, scale=SCALE)
                            if gs == 128 * j:
                                # diagonal block: causal-mask first 128 cols
                                nc.gpsimd.affine_select(
                                    out=pj[:, 0:128], in_=pj[:, 0:128],
                                    compare_op=mybir.AluOpType.is_ge, fill=0.0,
                                    base=0, pattern=[[1, 128]],
                                    channel_multiplier=-1)
                            for c in range(max(2 * qh, j // 4), 2 * qh + 2):
                                cs = max(512 * c, 128 * j)
                                w = 512 * (c + 1) - cs
                                nc.tensor.matmul(
                                    pav[:, ds(cs - q0, w)],
                                    (vsb[:, j, h, :]),
                                    (pj[:, ds(cs - gs, w)]),
                                    start=(j == 0),
                                    stop=(j == min(8 * (qh + 1) - 1, 4 * c + 3)))
                        # normalize: ot rows = pav[:64] / l, l = pav[64].
                        # reciprocal is lane-local (psum lane 64 -> sbuf
                        # lane 64); the hw broadcast ucode reads partition
                        # 0, so DMA the row there first.
                        qsl = ds(q0, 1024)
                        lr = lpool.tile([128, 1024], F32, tag="lr")
                        nc.vector.reciprocal(lr[ds(64, 1), :], pav[ds(DH, 1), :])
                        nc.sync.dma_start(lr[ds(0, 1), :], lr[ds(64, 1), :])
                        rb = lpool.tile([64, 1024], F32, tag="rb")
                        nc.gpsimd.partition_broadcast(rb[:], lr[ds(0, 1), :],
                                                      channels=64)
                        if h % 2 == 0:
                            nc.vector.tensor_mul(
                                ot[ds(0, 64), ht, qsl], pav[ds(0, DH), :], rb[:])
                        else:
                            ott = lpool.tile([64, 1024], BF16, tag="ott")
                            nc.vector.tensor_mul(ott[:], pav[ds(0, DH), :], rb[:])
                            nc.sync.dma_start(ot[ds(64, 64), ht, qsl], ott[:])

        # ---------------- out projection ---------------------------------
        if phases < 3:
            return
        with ExitStack() as ph:
            ypool = ph.enter_context(tc.tile_pool(name="ypool", bufs=4))
            psy = ph.enter_context(
                tc.tile_pool(name="psy", bufs=4, space="PSUM"))
            for tt in range(NT):
                ps = psy.tile([128, D], F32, tag="psy")
                for k in range(4):
                    for half in range(2):
                        nc.tensor.matmul(
                            ps[:, ts(half, 512)],
                            (ot[:, k, ts(tt, 128)]),
                            (wo_sb[k][:, ts(half, 512)]),
                            start=(k == 0), stop=(k == 3))
                ysb = ypool.tile([128, D], BF16, tag="y")
                # alternate the psum->sbuf drain between two engines so
                # the copy never gates the matmul pipeline (gpsimd cannot
                # read PSUM on hw)
                if tt % 2 == 0:
                    nc.scalar.copy(ysb[:], ps[:])
                else:
                    nc.vector.tensor_copy(ysb[:], ps[:])
                nc.sync.dma_start(y[ts(tt, 128), :], ysb[:])


# ---------------- host side ------------------------------------------------

def _rope_tables():
    # pair layout per 64-row head block, 16-granularity so the rope
    # swap is a within-quadrant stream_shuffle with mask i^16:
    # rows [32q+0..15] = even dims of pairs 16q..16q+15,
    # rows [32q+16..31] = odd dims of the same pairs
    i = np.arange(DH // 2, dtype=np.float32)
    thetas = np.power(np.float32(10000.0), -2.0 * (i - 1.0) / DH)
    vals = thetas[:, None].astype(np.float32) * \
        np.arange(S, dtype=np.float32)[None, :]
    cos32 = np.cos(vals).astype(np.float32)   # [32 pairs, S]
    sin32 = np.sin(vals).astype(np.float32)
    cc64 = np.concatenate([cos32[0:16], cos32[0:16],
                           cos32[16:32], cos32[16:32]], axis=0)
    ss64 = np.concatenate([sin32[0:16], -sin32[0:16],
                           sin32[16:32], -sin32[16:32]], axis=0)
    CC = np.tile(cc64, (2, 1))
    SSsw = np.tile(ss64, (2, 1))
    return np.ascontiguousarray(CC), np.ascontiguousarray(SSsw)


def _qk_col_perm(g):
    cols = []
    for m in range(4):
        for hh in (2 * m, 2 * m + 1):
            hg = HL * g + hh
            for q in (0, 1):
                for eo in (0, 1):
                    cols += [hg * DH + 2 * (16 * q + i) + eo
                             for i in range(16)]
    return np.array(cols)


_CACHE = {}


def _get_module(repeats=1):
    key = f"nc{repeats}"
    if key not in _CACHE:
        nc = bacc.Bacc("TRN2", target_bir_lowering=False, debug=False,
                       num_devices=8)
        build_kernel(nc, repeats=repeats)
        nc.compile()
        _CACHE[key] = nc
    return _CACHE[key]


def make_in_maps(x, Wqkv, Wout):
    b16 = mybir.dt.np(BF16)
    x = np.ascontiguousarray(np.asarray(x, np.float32))
    Wqkv = np.asarray(Wqkv, np.float32)
    Wout = np.asarray(Wout, np.float32)
    CC, SSsw = _rope_tables()
    cc_t = np.ascontiguousarray(CC).astype(b16)
    ssw_t = np.ascontiguousarray(SSsw).astype(b16)
    # tri[k, q] = 1 where q >= k (valid causal positions in a diagonal
    # 128x128 block)
    tri_t = np.ascontiguousarray(
        np.triu(np.ones((128, 128), np.float32))).astype(b16)

    shard = {}
    for g in range(2):
        perm = _qk_col_perm(g)
        vcols = np.arange(HL * g * DH, HL * (g + 1) * DH)
        shard[g] = dict(
            wq=np.ascontiguousarray(
                Wqkv[:, 0 * INNER:1 * INNER][:, perm]).astype(b16),
            wk=np.ascontiguousarray(
                Wqkv[:, 1 * INNER:2 * INNER][:, perm]).astype(b16),
            wv=np.ascontiguousarray(
                Wqkv[:, 2 * INNER:3 * INNER][:, vcols]).astype(b16),
            wo=np.ascontiguousarray(Wout[vcols, :]).astype(b16),
        )
    in_maps = []
    for c in range(8):
        b, g = c // 2, c % 2
        in_maps.append(dict(
            xT=np.ascontiguousarray(x[b].T).astype(b16),
            cc=cc_t, ssw=ssw_t, tri=tri_t, **shard[g]))
    return in_maps


def kernel(x, Wqkv, Wout, bout):
    bout = np.asarray(bout, np.float32)
    nc = _get_module()
    in_maps = make_in_maps(x, Wqkv, Wout)
    res = run_bass_kernel_spmd(nc, in_maps, core_ids=list(range(8)))
    ys = [np.asarray(r["y"], np.float32) for r in res.results]
    out = np.stack([(ys[2 * b] + ys[2 * b + 1]) * (1.0 / W_SCALE) + bout
                    for b in range(B)])
    return out.astype(np.float32)
